# revision 2
# baseline (speedup 1.0000x reference)
"""Trainium2 Bass kernel for nn_CustomConv2d: 3x3 conv, stride 1, pad 1.

Full shapes: x (32,128,56,56) f32, weight (256,128,3,3) f32, bias (256,) f32.
Output: (32,256,56,56) f32.

Strategy: data-parallel over batch (8 cores x 4 images). Compute in fp8e4
(e4m3) with DoubleRow perf mode: each PE matmul contracts TWO 128-deep k-tiles
at half the per-column cost of an fp32r matmul. Accuracy is recovered by
operand splitting: x ~= x8 + xr (both e4m3, residual quantized), and the
weight residual wr is applied on 4 of the 9 taps:

    out = x8*w8 + xr*w8 (9 taps each, paired same-tap)  -> 9 DoubleRow matmuls
        + x8*wr on taps {2,4,6,8}                       -> 2 DoubleRow matmuls

22 k-tiles vs the exact 18-k-tile-equivalent fp32r cost of 9 full-rate
matmuls: 11 * 0.5 = 5.5 column-cycles per output column vs 9.0 -> ~1.64x.
Measured max rel err vs the fp32 reference: ~1.6e-2 (< 2e-2 gate).

Layout: each image component is a flat zero-padded 58-col x 58-row fp8 plane,
so a conv window of N=464 (8 output rows x 58) is CONTIGUOUS; every tap is a
constant flat offset ky*58+kx and the DoubleRow pair dim is a constant stride
(3368 between the x8 and xr planes). 2 of each 58 output columns are garbage
(row wrap) and are dropped in the PSUM->SBUF bias-copy.
"""

import numpy as np
import ml_dtypes

import concourse.bass as bass
import concourse.mybir as mybir
import concourse.tile as tile
from concourse import bacc
from concourse.ap import AP
from concourse.bass_utils import run_bass_kernel_spmd

F8NP = ml_dtypes.float8_e4m3

N_CORES = 8
B = 32
B_LOC = B // N_CORES  # 4
CIN = 128
COUT = 256
H = W = 56
WP = 58  # padded row length
XPLANE = 3368  # 58*58 = 3364 rounded up: tap windows of the last tile overrun by 2
RCH = 8  # output rows per matmul tile
NCH = H // RCH  # 7
NT = RCH * WP  # 464: matmul output columns per tile (incl 2 garbage/row)
WTAPS = (2, 4, 6, 8)  # taps that get the weight-residual correction
NMM = 11  # DoubleRow matmuls per output tile

_NC_CACHE = None
LAST_RESULTS = None  # stashed BassKernelResults for test harness introspection

N_WARM = 40  # warmup matmuls bridging the p-state ramp until DMAs land

# output-column segments (q0, ncols) per image-half; the final image-half ends
# with a 1-row segment so the tail store chain is minimal
SEGMENTS = [(i * NT, NT) for i in range(NCH)]
SEGMENTS_LAST = [(i * NT, NT) for i in range(NCH - 1)] + [(2784, 406), (3190, 58)]


def _tap_off(t):
    return (t // 3) * WP + (t % 3)


def _build() -> bass.Bass:
    f8 = mybir.dt.float8e4
    f32 = mybir.dt.float32
    f32r = mybir.dt.float32r
    DR = mybir.MatmulPerfMode.DoubleRow
    nc = bacc.Bacc(None, target_bir_lowering=False)
    # x8 and xr planes per image, interleaved: [b][cin][comp][plane]
    x_d = nc.dram_tensor("x", [B_LOC, CIN, 2 * XPLANE], f8, kind="ExternalInput")
    # 11 stationary pairs per cout-half: [cin][half][mm][pair][cout128]
    wt_d = nc.dram_tensor("wt", [CIN, 2, NMM, 2, 128], f8, kind="ExternalInput")
    b_d = nc.dram_tensor("b", [2, 128], f32, kind="ExternalInput")
    y_d = nc.dram_tensor("y", [B_LOC, COUT, H * W], f32, kind="ExternalOutput")

    with tile.TileContext(nc) as tc:
        with tc.tile_pool(name="const", bufs=1) as cpool, tc.tile_pool(
            name="xp", bufs=B_LOC
        ) as xpool, tc.tile_pool(name="out", bufs=6) as opool, tc.tile_pool(
            name="ps", bufs=7, space="PSUM"
        ) as pspool:
            wtile = cpool.tile([CIN, 2, NMM, 2, 128], f8)
            xtiles = [
                xpool.tile([CIN, 2, XPLANE], f8, tag="xp", name=f"x{i}")
                for i in range(B_LOC)
            ]

            # PE warmup: dep-free f32 matmuls burn through the p-state ramp
            # while the first DMAs land (f32 needs no staging copy; small
            # tiles keep the bridge granular so it ends on time).
            wsrc0 = cpool.tile([128, 16], f32)
            nc.vector.memset(wsrc0[:], 0.0)
            wps = pspool.tile([16, 16], f32, tag="warmps", bufs=1)
            for _ in range(N_WARM):
                nc.tensor.matmul(wps[:], wsrc0[:], wsrc0[:], start=True, stop=True)

            # DMA issue order = criticality; weight/x chunks sized so the
            # first tiles' operands land just-in-time through the serial
            # DMA-engine pipe.
            x0v = x_d[0].rearrange("c (p n) -> c p n", p=2)
            XCH = (600, 1160, 1976, XPLANE)
            nc.sync.dma_start(xtiles[0][:, :, 0 : XCH[0]], x0v[:, :, 0 : XCH[0]])
            nc.sync.dma_start(wtile[:, 0, 0:2], wt_d[:, 0, 0:2])
            nc.sync.dma_start(wtile[:, 0, 2:6], wt_d[:, 0, 2:6])
            nc.sync.dma_start(
                xtiles[0][:, :, XCH[0] : XCH[1]], x0v[:, :, XCH[0] : XCH[1]]
            )
            nc.sync.dma_start(wtile[:, 0, 6:NMM], wt_d[:, 0, 6:NMM])
            nc.sync.dma_start(
                xtiles[0][:, :, XCH[1] : XCH[2]], x0v[:, :, XCH[1] : XCH[2]]
            )
            nc.sync.dma_start(
                xtiles[0][:, :, XCH[2] : XCH[3]], x0v[:, :, XCH[2] : XCH[3]]
            )
            nc.sync.dma_start(wtile[:, 1], wt_d[:, 1])
            btile = cpool.tile([128, 2], f32)
            nc.scalar.dma_start(btile[:], b_d[:].rearrange("t p -> p t"))
            for b in range(1, B_LOC):
                nc.sync.dma_start(xtiles[b][:], x_d[b])

            for b in range(B_LOC):
                xt = xtiles[b][:]  # [128, 2, XPLANE]
                pstride = xt.ap[0][0]
                xbase = xt.offset
                for t in range(2):
                    last_bt = (b == B_LOC - 1) and (t == 1)
                    segs = SEGMENTS_LAST if last_bt else SEGMENTS
                    for r, (q0, ncols) in enumerate(segs):
                        psf = pspool.tile([128, NT], f32, name="ps")
                        ps = psf[:, :ncols]
                        # 9 same-tap pairs: (x8 win | xr win), lhsT (w8_t, w8_t)
                        for tap in range(9):
                            rhs = AP(
                                xt.tensor,
                                xbase + q0 + _tap_off(tap),
                                [[pstride, 128], [XPLANE, 2], [1, ncols]],
                            )
                            nc.tensor.matmul(
                                ps[:],
                                wtile[:, t, tap],
                                rhs,
                                start=(tap == 0),
                                stop=False,
                                perf_mode=DR,
                            )
                        # 2 wr pairs: (x8 win_a | x8 win_b), lhsT (wr_a, wr_b)
                        for i in range(2):
                            ta, tb = WTAPS[2 * i], WTAPS[2 * i + 1]
                            rhs = AP(
                                xt.tensor,
                                xbase + q0 + _tap_off(ta),
                                [
                                    [pstride, 128],
                                    [_tap_off(tb) - _tap_off(ta), 2],
                                    [1, ncols],
                                ],
                            )
                            nc.tensor.matmul(
                                ps[:],
                                wtile[:, t, 9 + i],
                                rhs,
                                start=False,
                                stop=(i == 1),
                                perf_mode=DR,
                            )
                        # drop the 2 garbage cols per row while adding bias
                        nr = ncols // WP
                        psv = ps[:].rearrange("p (h w) -> p h w", h=nr)[:, :, 0:W]
                        ochunk = opool.tile([128, nr * W], f32, tag="ochunk", name=f"oc{nr}")
                        ov = ochunk[:].rearrange("p (h w) -> p h w", h=nr)
                        if last_bt and ncols == 406:
                            # tail: split the copy across both engines so the
                            # final stores' data is ready as early as possible
                            nc.scalar.activation(
                                ov[:, 0:4],
                                psv[:, 0:4],
                                mybir.ActivationFunctionType.Identity,
                                bias=btile[:, t : t + 1],
                            )
                            nc.vector.tensor_scalar_add(
                                ov[:, 4:nr], psv[:, 4:nr], btile[:, t : t + 1]
                            )
                        elif r % 2 == 0:
                            nc.scalar.activation(
                                ov,
                                psv,
                                mybir.ActivationFunctionType.Identity,
                                bias=btile[:, t : t + 1],
                            )
                        else:
                            nc.vector.tensor_scalar_add(ov, psv, btile[:, t : t + 1])
                        r0 = q0 // WP
                        qeng = nc.scalar if (last_bt and ncols == 58) else nc.sync
                        qeng.dma_start(
                            y_d[b, t * 128 : (t + 1) * 128, r0 * W : r0 * W + nr * W],
                            ochunk[:],
                        )
    nc.finalize()
    return nc


def kernel(x, weight, bias, approximate):
    """Full (unsharded) conv2d. `approximate` only selects the HW approximation
    level in the original module; the exact-math output is independent of it."""
    global _NC_CACHE, LAST_RESULTS
    x = np.ascontiguousarray(x, dtype=np.float32)
    weight = np.ascontiguousarray(weight, dtype=np.float32)
    bias = np.ascontiguousarray(bias, dtype=np.float32)

    # fp8 split of x: x ~= x8 + xr, both e4m3, laid out as flat zero-padded
    # 58x58 planes (row 0 / row 57 / col 0 / col 57 are zero padding)
    x8 = x.astype(F8NP)
    xr = (x - x8.astype(np.float32)).astype(F8NP)
    xplanes = np.zeros((B, CIN, 2, 58, 58), F8NP)
    xplanes[:, :, 0, 1 : H + 1, 1 : W + 1] = x8
    xplanes[:, :, 1, 1 : H + 1, 1 : W + 1] = xr
    xflat = np.zeros((B, CIN, 2, XPLANE), F8NP)
    xflat[:, :, :, : 58 * 58] = xplanes.reshape(B, CIN, 2, 58 * 58)
    xflat = xflat.reshape(B, CIN, 2 * XPLANE)

    # weights: w8 + wr(4 taps); stationary pairs [cin][half][mm][pair][cout128]
    w8 = weight.astype(F8NP)
    wr = (weight - w8.astype(np.float32)).astype(F8NP)
    wpack = np.zeros((CIN, 2, NMM, 2, 128), F8NP)
    for t in range(2):
        wh8 = w8[t * 128 : (t + 1) * 128]  # [128cout, 128cin, 3, 3]
        whr = wr[t * 128 : (t + 1) * 128]
        for tap in range(9):
            wtap = wh8[:, :, tap // 3, tap % 3].T  # [cin, cout]
            wpack[:, t, tap, 0] = wtap
            wpack[:, t, tap, 1] = wtap
        for i in range(2):
            ta, tb = WTAPS[2 * i], WTAPS[2 * i + 1]
            wpack[:, t, 9 + i, 0] = whr[:, :, ta // 3, ta % 3].T
            wpack[:, t, 9 + i, 1] = whr[:, :, tb // 3, tb % 3].T

    b2 = bias.reshape(2, 128)

    if _NC_CACHE is None:
        _NC_CACHE = _build()
    nc = _NC_CACHE

    in_maps = [
        {"x": xflat[c * B_LOC : (c + 1) * B_LOC], "wt": wpack, "b": b2}
        for c in range(N_CORES)
    ]
    try:
        res = run_bass_kernel_spmd(nc, in_maps, core_ids=list(range(N_CORES)))
    except Exception:
        # transient device-acquisition races recover on retry
        import time as _time

        _time.sleep(5.0)
        res = run_bass_kernel_spmd(nc, in_maps, core_ids=list(range(N_CORES)))
    LAST_RESULTS = res
    out = np.concatenate([r["y"] for r in res.results], axis=0)
    return out.reshape(B, COUT, H, W)
